# revision 2
# baseline (speedup 1.0000x reference)
"""MoE routing kernel for TRN2, SPMD over 8 NeuronCores — v3.

Problem (per reference):
  x = mean(hidden_states, axis=1)                  # [B, H]
  scores = x @ gate_w + gate_b                     # [B, E]
  weights, sel = top_k(scores, 2)
  all_out = einsum('bh,eho->beo', x, expert_w) + expert_b
  out = sum(weights * all_out[b, sel], axis=1)     # [B, H]

Shapes: B=2048, S=256, H=1024, E=8, TOPK=2, fp32.  Data-parallel over
batch: 256 tokens/core, gate + expert weights replicated.

Measured device characteristics (this axon-tunneled TRN2 setup):
  - DMA throughput is governed by a latency-bandwidth product (~40 us
    effective per-DMA latency): rate ~= in-flight bytes / 40 us.  Deep
    buffering of small DMAs beats big DMAs and extra rings.
  - fp32 DVE ops run at 1 elem/cycle/lane (123 G elem/s) — too slow for
    the 67M-element mean; fp32 matmul is 4 cycles/row, f32r is 1.

Design:
  1. One deep SBUF pool (~20 x [128, 2048] fp32 tiles) holds the stream:
     256 hidden_states chunks ([128 tok, 2 s, 1024 h], 1 MB each), then
     32 expert_w "pair" tiles ([128 h_in, 2*1024], 1 MB) rotate through
     the same pool, so the full in-flight budget applies to both phases.
  2. Mean over S on the PE: identity/S-stationary f32r matmuls accumulate
     each s-slice into PSUM [128 tok, 512 h] (~220 us << DMA time).
  3. x -> xT via PE transposes; gate + top-2 -> per-token combine weights
     m (raw scores masked to top-2); out = m @ expert_b + sum_e m_e * (x @ W_e)
     with per-expert PSUM groups and DVE scalar_tensor_tensor combines.
  4. Everything f32r (~2e-4 matmul rel err); top-2 ties break as in ref.
"""

import numpy as np

B, S, H, E = 2048, 256, 1024, 8
N_CORES = 8
B_LOC = B // N_CORES          # 256 tokens per core
N_TT = B_LOC // 128           # 2 token-tiles of 128
CS = 2                        # s-values per hs chunk -> [128, 2048] 1MB DMAs
KC = H // 128                 # 8 contraction chunks
NCH = H // 512                # 2 output column chunks
W_PAIRS = E * KC // 2         # 32 expert-weight pair tiles

_compiled = None


def _build(reps=1, mode="full", cs=CS, bufs=20, rings=("sync", "scalar")):
    import concourse.bacc as bacc
    import concourse.mybir as mybir
    import concourse.tile as tile
    from concourse.masks import make_identity

    fp32 = mybir.dt.float32
    f32r = mybir.dt.float32r
    n_sc = S // cs
    nc = bacc.Bacc("TRN2", target_bir_lowering=False, debug=False,
                   num_devices=N_CORES)

    hs = nc.dram_tensor("hidden_states", [B_LOC, S, H], f32r,
                        kind="ExternalInput").ap()
    gate_w = nc.dram_tensor("gate_w", [H, E], f32r, kind="ExternalInput").ap()
    gate_b = nc.dram_tensor("gate_b", [E], f32r, kind="ExternalInput").ap()
    expert_w = nc.dram_tensor("expert_w", [E, H, H], f32r,
                              kind="ExternalInput").ap()
    expert_b = nc.dram_tensor("expert_b", [E, H], f32r,
                              kind="ExternalInput").ap()
    out = nc.dram_tensor("out", [B_LOC, H], fp32, kind="ExternalOutput").ap()

    with tile.TileContext(nc) as tc:
        with (
            tc.tile_pool(name="str", bufs=bufs) as str_pool,
            tc.tile_pool(name="acc", bufs=1) as acc_pool,
            tc.tile_pool(name="small", bufs=1) as small_pool,
            tc.tile_pool(name="top2", bufs=1) as top2_pool,
            tc.tile_pool(name="px", bufs=1, space="PSUM") as px_pool,
            tc.tile_pool(name="ps", bufs=4, space="PSUM") as ps_pool,
            tc.tile_pool(name="psmall", bufs=2, space="PSUM") as psmall_pool,
        ):
            # --- constants / small inputs ---
            identity = small_pool.tile([128, 128], fp32, tag="ident")
            make_identity(nc, identity[:])
            ident_s = small_pool.tile([128, 128], f32r, tag="idents")
            nc.vector.tensor_scalar_mul(ident_s[:], identity[:], 1.0 / S)
            ones_f = small_pool.tile([1, 128], fp32, tag="onesf")
            nc.gpsimd.memset(ones_f[:], 1.0)
            ones_row = small_pool.tile([1, 128], f32r, tag="ones")
            nc.vector.tensor_copy(out=ones_row[:], in_=ones_f[:])

            gw_s = small_pool.tile([128, KC * E], f32r, tag="gw")
            for kc in range(KC):
                nc.gpsimd.dma_start(
                    out=gw_s[:, kc * E:(kc + 1) * E],
                    in_=gate_w[kc * 128:(kc + 1) * 128, :])
            gb_s = small_pool.tile([1, E], f32r, tag="gb")
            nc.gpsimd.dma_start(out=gb_s[:], in_=gate_b[None, :])
            eb_s = small_pool.tile([E, H], f32r, tag="eb")
            nc.gpsimd.dma_start(out=eb_s[:], in_=expert_b[:, :])

            def body():
                xT = []        # [128 h, B_LOC tok] f32r, per kc
                for kc in range(KC):
                    xT.append(acc_pool.tile([128, B_LOC], f32r,
                                            tag=f"xt{kc}", name=f"xt{kc}"))
                m_tiles = []
                out_accs = []
                ring_i = [0]

                def next_ring():
                    eng = getattr(nc, rings[ring_i[0] % len(rings)])
                    ring_i[0] += 1
                    return eng

                def stream_tt(tt):
                    px = [px_pool.tile([128, 512], fp32, tag=f"px{nh}",
                                       name=f"px{nh}") for nh in range(NCH)]
                    for sc in range(n_sc):
                        chunk = str_pool.tile([128, cs * H], f32r, tag="s",
                                              name="chunk")
                        next_ring().dma_start(
                            out=chunk[:].rearrange("p (s h) -> p s h", s=cs),
                            in_=hs[tt * 128:(tt + 1) * 128,
                                   sc * cs:(sc + 1) * cs, :])
                        if mode == "dma":
                            continue
                        for j in range(cs):
                            for nh in range(NCH):
                                nc.tensor.matmul(
                                    px[nh][:],
                                    ident_s[:],
                                    chunk[:, j * H + nh * 512:
                                          j * H + (nh + 1) * 512],
                                    start=(sc == 0 and j == 0),
                                    stop=(sc == n_sc - 1 and j == cs - 1))
                    return px

                def finish_tt(tt, px):
                    x_sb = acc_pool.tile([128, H], fp32, tag="x", name="x_sb")
                    for nh in range(NCH):
                        nc.vector.tensor_copy(
                            out=x_sb[:, nh * 512:(nh + 1) * 512],
                            in_=px[nh][:])
                    for kc in range(KC):
                        pt = psmall_pool.tile([128, 128], fp32, tag="pt",
                                              name="pt")
                        nc.tensor.transpose(
                            pt[:],
                            x_sb[:, kc * 128:(kc + 1) * 128],
                            identity[:])
                        nc.vector.tensor_copy(
                            out=xT[kc][:, tt * 128:(tt + 1) * 128], in_=pt[:])

                    ps_sc = psmall_pool.tile([128, 128], fp32, tag="pt",
                                             name="psc")
                    for kc in range(KC):
                        nc.tensor.matmul(
                            ps_sc[:, :E],
                            xT[kc][:, tt * 128:(tt + 1) * 128],
                            gw_s[:, kc * E:(kc + 1) * E],
                            start=(kc == 0), stop=False)
                    nc.tensor.matmul(ps_sc[:, :E], ones_row[:],
                                     gb_s[:],
                                     start=False, stop=True)
                    s_t = top2_pool.tile([128, E], fp32, tag=f"s{tt}")
                    nc.vector.tensor_copy(out=s_t[:], in_=ps_sc[:, :E])
                    max1 = top2_pool.tile([128, 1], fp32, tag=f"mx1{tt}")
                    nc.vector.tensor_reduce(
                        max1[:], s_t[:], mybir.AxisListType.X,
                        mybir.AluOpType.max)
                    ge1 = top2_pool.tile([128, E], fp32, tag=f"ge1{tt}")
                    nc.vector.tensor_scalar(
                        ge1[:], s_t[:], max1[:], None, mybir.AluOpType.is_ge)
                    masked = top2_pool.tile([128, E], fp32, tag=f"msk{tt}")
                    nc.vector.scalar_tensor_tensor(
                        out=masked[:], in0=ge1[:], scalar=-1e30, in1=s_t[:],
                        op0=mybir.AluOpType.mult, op1=mybir.AluOpType.add)
                    max2 = top2_pool.tile([128, 1], fp32, tag=f"mx2{tt}")
                    nc.vector.tensor_reduce(
                        max2[:], masked[:], mybir.AxisListType.X,
                        mybir.AluOpType.max)
                    ge2 = top2_pool.tile([128, E], fp32, tag=f"ge2{tt}")
                    nc.vector.tensor_scalar(
                        ge2[:], s_t[:], max2[:], None, mybir.AluOpType.is_ge)
                    m_t = top2_pool.tile([128, E], fp32, tag=f"m{tt}")
                    nc.vector.tensor_mul(m_t[:], s_t[:], ge2[:])
                    m_tiles.append(m_t)
                    pmT = psmall_pool.tile([128, 128], fp32, tag="pt",
                                           name="pmT")
                    nc.tensor.transpose(pmT[:E, :], m_t[:], identity[:])
                    mT = top2_pool.tile([E, 128], f32r, tag=f"mT{tt}")
                    nc.vector.tensor_copy(out=mT[:], in_=pmT[:E, :])

                    oa = acc_pool.tile([128, H], fp32, tag=f"oa{tt}",
                                       name=f"oa{tt}")
                    for nh in range(NCH):
                        pb = ps_pool.tile([128, 512], fp32, tag="ps",
                                          name="pb")
                        nc.tensor.matmul(
                            pb[:], mT[:],
                            eb_s[:, nh * 512:(nh + 1) * 512],
                            start=True, stop=True)
                        nc.vector.tensor_copy(
                            out=oa[:, nh * 512:(nh + 1) * 512], in_=pb[:])
                    out_accs.append(oa)

                # ---- schedule ----
                px0 = stream_tt(0)
                if mode != "dma":
                    finish_tt(0, px0)
                px1 = stream_tt(1)

                # W pair tiles rotate through the same stream pool: pair p
                # covers expert e=p//4 rows [2q*128, (2q+2)*128), q=p%4.
                w_pairs = []
                for p in range(W_PAIRS):
                    e, q = divmod(p, 4)
                    wt = str_pool.tile([128, cs * H], f32r, tag="s",
                                       name="wpair")
                    next_ring().dma_start(
                        out=wt[:].rearrange("p (t h) -> p t h", t=2),
                        in_=expert_w[e, q * 256:(q + 1) * 256, :]
                        .rearrange("(t p) h -> p t h", t=2))
                    w_pairs.append(wt)

                if mode == "dma":
                    z = acc_pool.tile([128, H], fp32, tag="oa0", name="z")
                    nc.vector.memset(z[:], 0.0)
                    for tt in range(N_TT):
                        nc.sync.dma_start(
                            out=out[tt * 128:(tt + 1) * 128, :], in_=z[:])
                    return

                finish_tt(1, px1)
                if mode == "mean":
                    for tt in range(N_TT):
                        nc.sync.dma_start(
                            out=out[tt * 128:(tt + 1) * 128, :],
                            in_=out_accs[tt][:])
                    return

                for e in range(E):
                    for tt in range(N_TT):
                        for nh in range(NCH):
                            ps = ps_pool.tile([128, 512], fp32, tag="ps",
                                              name="ps")
                            for kc in range(KC):
                                wt = w_pairs[e * 4 + kc // 2]
                                half = kc % 2
                                nc.tensor.matmul(
                                    ps[:],
                                    xT[kc][:, tt * 128:(tt + 1) * 128],
                                    wt[:, half * H + nh * 512:
                                       half * H + (nh + 1) * 512],
                                    start=(kc == 0), stop=(kc == KC - 1))
                            sl = out_accs[tt][:, nh * 512:(nh + 1) * 512]
                            nc.vector.scalar_tensor_tensor(
                                out=sl, in0=ps[:],
                                scalar=m_tiles[tt][:, e:e + 1],
                                in1=sl, op0=mybir.AluOpType.mult,
                                op1=mybir.AluOpType.add)
                    if e == E - 1:
                        for tt in range(N_TT):
                            nc.sync.dma_start(
                                out=out[tt * 128:(tt + 1) * 128, :],
                                in_=out_accs[tt][:])

            if reps == 1:
                body()
            else:
                with tc.For_i(0, reps, 1):
                    body()

    nc.compile()
    return nc


def _get_compiled():
    global _compiled
    if _compiled is None:
        _compiled = _build()
    return _compiled


def kernel(**inputs):
    from concourse.bass_utils import run_bass_kernel_spmd

    hs = np.ascontiguousarray(np.asarray(inputs["hidden_states"],
                                         dtype=np.float32))
    gw = np.ascontiguousarray(np.asarray(inputs["gate_w"], dtype=np.float32))
    gb = np.ascontiguousarray(np.asarray(inputs["gate_b"], dtype=np.float32))
    ew = np.ascontiguousarray(np.asarray(inputs["expert_w"],
                                         dtype=np.float32))
    eb = np.ascontiguousarray(np.asarray(inputs["expert_b"],
                                         dtype=np.float32))

    nc = _get_compiled()
    in_maps = []
    for i in range(N_CORES):
        in_maps.append({
            "hidden_states": hs[i * B_LOC:(i + 1) * B_LOC],
            "gate_w": gw,
            "gate_b": gb,
            "expert_w": ew,
            "expert_b": eb,
        })
    res = run_bass_kernel_spmd(nc, in_maps, list(range(N_CORES)), trace=False)
    return np.concatenate([res.results[i]["out"] for i in range(N_CORES)],
                          axis=0)
